# revision 44
# baseline (speedup 1.0000x reference)
"""GAT (bipartite GATConv + mean-pool + 2 FC) on 8 Trainium2 NeuronCores.

Strategy: edges are sharded per destination node; destination nodes are
dealt round-robin (degree-sorted) across the 8 cores so the segment softmax
is fully local to a core.  Per the sharding hint each device holds its edge
shard with the source-node features replicated into matmul-ready per-edge
layout (host does only index manipulation / np.take layout; every model
FLOP runs on device):

  Phase A2: a_t = x_t @ (W att_dst) for this core's dst nodes (PE).
  Phase B: dst nodes are processed in tiles of 128 (one node per partition,
           nodes degree-sorted so tiles have uniform run lengths L).  The
           per-edge source features arrive as bf16 lhsT tiles [128, L/2*128]
           (slot pair 2j/2j+1 stacked as two K=64 halves); one K=128 matmul
           against a block-diagonal [wfold|0 / 0|wfold] rhs computes BOTH
           slots' rows [h_s (36, c-major) | a_s (3) | pad] into PSUM.  ACT
           casts them to bf16 SBUF; E = exp(leaky_relu(a_s+a_t)) =
           max(exp(z), exp(0.2 z)) via two ACT Exp passes over a DVE-added
           z.  The weighted message sum runs in bf16 on DVE's packed 2x
           path: W columns are stored c-major/h-minor so the E broadcast's
           innermost dim is the packed head dim, and the slot reduction is
           two packed tree-add levels plus a short tensor_reduce tail.
           Batch pooling is a PE matmul against host-shipped one-hot
           columns, accumulated over all tiles in PSUM; the final fc1@fc3
           contraction and count division happen on host partials.
           Pad slots carry x=0 => h=0, a_s=0; their exp(leaky_relu(a_t))
           denominator contribution is subtracted exactly via a
           host-precomputed pad-count correction.

Device-side per-edge gathers are avoided entirely: one [P,1]-offset
indirect-DMA gather costs ~1.1us of SWDGE descriptor generation on the Pool
engine (994ns fixed + 0.34ns/desc, 128 descriptors max per instruction) and
the batched-gather ucode (InstDMAGatherAnt etc.) is excluded from bedrock
images, so any gather-based design is floored at ~3.3ms/core.  Sequential
streaming of the pre-laid-out edge shard runs at DMA bandwidth instead.

HW pitfalls (probed): matmuls that switch PE row groups (partition-offset
lhsT/rhs) within one PSUM tile crash the device (the K=128 block-diagonal
formulation sidesteps row groups); Pool-engine TensorTensor is rejected by
this lowering; softmax denominators accumulated from bf16 exps lose ~6x
final accuracy (E stays f32; bf16 is cast only for the message multiply).
"""

import numpy as np
import ml_dtypes

import concourse.bacc as bacc
import concourse.tile as tile
from concourse import mybir
from concourse.bass_utils import run_bass_kernel_spmd

F32 = mybir.dt.float32
BF16 = mybir.dt.bfloat16

N_CORES = 8
P = 128
HEADS = 3
CH = 12
HC = HEADS * CH          # 36
ROW = HC + 4             # matmul output row: 36 h | 3 a_s | 1 pad = 40
ROW2 = 2 * ROW           # block-diagonal pair output
NEG_SLOPE = 0.2
GP = 6                   # slot pairs per PSUM tile (6*80 = 480 f32 <= 512)

_nc_cache = {}


def _build_nc(in_dim, n_dst_tiles, L_list, half_tot, n_xt_cols, groups):
    key = (in_dim, n_dst_tiles, tuple(L_list), half_tot, n_xt_cols, tuple(groups))
    if key in _nc_cache:
        return _nc_cache[key]

    nc = bacc.Bacc("TRN2", target_bir_lowering=False, debug=False)
    d_xe = nc.dram_tensor("xe", [2 * in_dim, half_tot * P], BF16, kind="ExternalInput")
    d_xt = nc.dram_tensor("xt_t", [in_dim, n_xt_cols], F32, kind="ExternalInput")
    d_pc = nc.dram_tensor("padc4", [P, n_dst_tiles * 4], F32, kind="ExternalInput")
    d_oh = nc.dram_tensor("oh", [P, n_dst_tiles * P], F32, kind="ExternalInput")
    d_wf = nc.dram_tensor("wfbd", [2 * in_dim, ROW2], BF16, kind="ExternalInput")
    d_wt = nc.dram_tensor("wat", [in_dim, 4], F32, kind="ExternalInput")
    d_bb = nc.dram_tensor("biasb", [P, HC], F32, kind="ExternalInput")
    d_q = nc.dram_tensor("q_out", [P, HC], F32, kind="ExternalOutput")

    with tile.TileContext(nc) as tc:
        with tc.tile_pool(name="const", bufs=1) as cpool, \
             tc.tile_pool(name="xload", bufs=2) as xpool, \
             tc.tile_pool(name="gat", bufs=3) as gpool, \
             tc.tile_pool(name="work", bufs=3) as wpool, \
             tc.tile_pool(name="msg", bufs=2) as mpool, \
             tc.tile_pool(name="psA", bufs=4, space="PSUM") as psA, \
             tc.tile_pool(name="psB", bufs=1, space="PSUM") as psB, \
             tc.tile_pool(name="psT", bufs=2, space="PSUM") as psT:

            # ---- constants into SBUF ----
            t_wf = cpool.tile([2 * in_dim, ROW2], BF16)
            nc.sync.dma_start(t_wf[:], d_wf[:])
            t_wt = cpool.tile([in_dim, 4], F32)
            nc.sync.dma_start(t_wt[:], d_wt[:])
            t_bb = cpool.tile([P, HC], F32)
            nc.sync.dma_start(t_bb[:], d_bb[:])
            t_pc = cpool.tile([P, n_dst_tiles * 4], F32)
            nc.sync.dma_start(t_pc[:], d_pc[:])

            # ---- phase B: tiles processed in groups sharing L (the few
            # high-degree tiles run solo; the rest in groups of 4).
            # Software-pipelined: group i's DMA/matmul/copy/z/exp stage (A)
            # is emitted before group i-1's softmax/message stage (B), so
            # the in-order DVE stream has group i-1's heavy message work to
            # run while ACT computes group i's exponentials. ----
            ps_q = psB.tile([P, HC], F32, space="PSUM", tag="q")
            state = {}
            off_h = [0]

            def emit_A(idx):
                t0, gs = groups[idx]
                L = L_list[t0]        # shared within group, multiple of 4
                Lh = L // 2
                GL = gs * L
                xe_sb = xpool.tile([2 * in_dim, gs * Lh * P], BF16, tag="xe")
                nc.sync.dma_start(
                    xe_sb[:], d_xe[:, off_h[0] * P:(off_h[0] + gs * Lh) * P])
                off_h[0] += gs * Lh

                # a_t for this group's tiles (interleaved A2; per-group
                # x_t slice so the first group isn't gated on a monolithic
                # x_t load)
                xt_g = wpool.tile([in_dim, gs * P], F32, tag="xt")
                nc.sync.dma_start(xt_g[:], d_xt[:, t0 * P:(t0 + gs) * P])
                t_atg = wpool.tile([P, 4 * gs], F32, tag="at")
                psa2 = psT.tile([P, 4 * gs], F32, space="PSUM", tag="psat")
                for j in range(gs):
                    nc.tensor.matmul(
                        psa2[:, j * 4:(j + 1) * 4],
                        lhsT=xt_g[:, j * P:(j + 1) * P],
                        rhs=t_wt[:], start=True, stop=True)
                nc.scalar.copy(t_atg[:], psa2[:])

                # per-edge rows via PE: one K=128 matmul per slot PAIR
                g = gpool.tile([P, GL * ROW], BF16, tag="G")
                for st in range(gs):
                    for h0 in range(0, Lh, GP):
                        nh = min(GP, Lh - h0)
                        ps = psA.tile([P, GP * ROW2], F32, space="PSUM", tag="psa")
                        for j in range(nh):
                            nc.tensor.matmul(
                                ps[:, j * ROW2:(j + 1) * ROW2],
                                lhsT=xe_sb[:, (st * Lh + h0 + j) * P:
                                           (st * Lh + h0 + j + 1) * P],
                                rhs=t_wf[:],
                                start=True, stop=True)
                        nc.scalar.copy(
                            g[:, (st * Lh + h0) * ROW2:
                              (st * Lh + h0 + nh) * ROW2],
                            ps[:, :nh * ROW2])

                g4 = g[:].rearrange("p (s l c) -> p s l c", s=gs, c=ROW)

                # z = a_s + a_t  (layout (s, l, h)); exps on ACT
                tZ = wpool.tile([P, GL * HEADS], F32, tag="Z")
                Z4 = tZ[:].rearrange("p (s l h) -> p s l h", s=gs, h=HEADS)
                at_b = (t_atg[:].rearrange("p (s h) -> p s h", h=4)
                        [:, :, 0:HEADS]
                        .unsqueeze(2).to_broadcast((P, gs, L, HEADS)))
                nc.vector.tensor_tensor(
                    out=Z4[:], in0=g4[:, :, :, HC:HC + HEADS], in1=at_b,
                    op=mybir.AluOpType.add)
                tE = wpool.tile([P, GL * HEADS], F32, tag="E")
                tT = wpool.tile([P, GL * HEADS], F32, tag="T")
                nc.scalar.activation(
                    tT[:], tZ[:], mybir.ActivationFunctionType.Exp)
                nc.scalar.activation(
                    tE[:], tZ[:], mybir.ActivationFunctionType.Exp,
                    scale=NEG_SLOPE)
                state[idx] = (t0, gs, L, GL, g, tE, tT, t_atg)

            def emit_B(idx):
                t0, gs, L, GL, g, tE, tT, t_atg = state.pop(idx)
                nc.vector.tensor_tensor(
                    out=tE[:], in0=tE[:], in1=tT[:], op=mybir.AluOpType.max)
                tEb = wpool.tile([P, GL * HEADS], BF16, tag="Eb")
                nc.scalar.copy(tEb[:], tE[:])

                # denominators + pad correction + reciprocal
                t_den = wpool.tile([P, 4 * gs], F32, tag="den")
                t_rec = wpool.tile([P, 4 * gs], F32, tag="rec")
                nc.vector.memset(t_den[:], 1.0)
                nc.vector.tensor_reduce(
                    out=t_den[:].rearrange("p (s h) -> p s h", h=4)
                    [:, :, 0:HEADS],
                    in_=tE[:].rearrange("p (s l h) -> p s l h", s=gs, h=HEADS)
                    .transpose([0, 1, 3, 2]),
                    axis=mybir.AxisListType.X, op=mybir.AluOpType.add)
                # pad correction: cor = padc * max(exp(a_t), exp(0.2 a_t))
                t_c2 = wpool.tile([P, 4 * gs], F32, tag="c2")
                t_c3 = wpool.tile([P, 4 * gs], F32, tag="c3")
                nc.vector.tensor_scalar_mul(t_c2[:], t_atg[:], NEG_SLOPE)
                nc.scalar.activation(
                    t_c3[:], t_c2[:], mybir.ActivationFunctionType.Exp)
                nc.scalar.activation(
                    t_c2[:], t_atg[:], mybir.ActivationFunctionType.Exp)
                nc.vector.tensor_tensor(
                    out=t_c2[:], in0=t_c2[:], in1=t_c3[:],
                    op=mybir.AluOpType.max)
                nc.vector.tensor_tensor(
                    out=t_c2[:], in0=t_c2[:],
                    in1=t_pc[:, t0 * 4:(t0 + gs) * 4],
                    op=mybir.AluOpType.mult)
                dv = (t_den[:].rearrange("p (s h) -> p s h", h=4)
                      [:, :, 0:HEADS])
                cview = (t_c2[:].rearrange("p (s h) -> p s h", h=4)
                         [:, :, 0:HEADS])
                nc.vector.tensor_tensor(
                    out=dv, in0=dv, in1=cview, op=mybir.AluOpType.subtract)
                nc.vector.tensor_scalar_max(t_den[:], t_den[:], 1e-30)
                nc.vector.reciprocal(t_rec[:], t_den[:])

                # weighted messages M = e * h (bf16; c-major h block),
                # one multiply for the whole group ((s,l) merged), then two
                # group-wide pairwise tree-add levels + one reduce tail
                tM = mpool.tile([P, GL * HC], BF16, tag="M")
                tU = wpool.tile([P, gs * HC], F32, tag="U")
                M4v = tM[:].rearrange("p (q c h) -> p q c h", c=CH, h=HEADS)
                e_b = (tEb[:].rearrange("p (q h) -> p q h", h=HEADS)
                       .unsqueeze(2).to_broadcast((P, GL, CH, HEADS)))
                gh = (g[:].rearrange("p (q c) -> p q c", c=ROW)[:, :, 0:HC]
                      .rearrange("p q (c h) -> p q c h", h=HEADS))
                nc.vector.tensor_tensor(
                    out=M4v[:], in0=gh, in1=e_b, op=mybir.AluOpType.mult)
                for n in (L // 2, L // 4):
                    sv = tM[:].rearrange("p (s q) -> p s q", s=gs)
                    src = (sv[:, :, :2 * n * HC]
                           .rearrange("p s (n two c) -> p s n two c",
                                      two=2, c=HC))
                    dstv = (sv[:, :, :n * HC]
                            .rearrange("p s (n c) -> p s n c", c=HC))
                    nc.vector.tensor_tensor(
                        out=dstv, in0=src[:, :, :, 0, :],
                        in1=src[:, :, :, 1, :], op=mybir.AluOpType.add)
                nc.vector.tensor_reduce(
                    out=tU[:].rearrange("p (s c) -> p s c", s=gs),
                    in_=tM[:].rearrange("p (s q) -> p s q", s=gs)
                    [:, :, :(L // 4) * HC]
                    .rearrange("p s (n c) -> p s n c", c=HC)
                    .transpose([0, 1, 3, 2]),
                    axis=mybir.AxisListType.X, op=mybir.AluOpType.add)

                # V = relu(U / denom + bias); relu on ACT
                tV = wpool.tile([P, gs * HC], F32, tag="V")
                rec_b = (t_rec[:].rearrange("p (s h) -> p s h", h=4)
                         [:, :, 0:HEADS].unsqueeze(2)
                         .to_broadcast((P, gs, CH, HEADS)))
                nc.vector.tensor_tensor(
                    out=tV[:].rearrange("p (s c h) -> p s c h", c=CH, h=HEADS),
                    in0=tU[:].rearrange("p (s c h) -> p s c h", c=CH, h=HEADS),
                    in1=rec_b, op=mybir.AluOpType.mult)
                bb_b = t_bb[:].unsqueeze(1).to_broadcast((P, gs, HC))
                nc.vector.tensor_tensor(
                    out=tV[:].rearrange("p (s c) -> p s c", s=gs),
                    in0=tV[:].rearrange("p (s c) -> p s c", s=gs),
                    in1=bb_b, op=mybir.AluOpType.add)
                nc.scalar.activation(
                    tV[:], tV[:], mybir.ActivationFunctionType.Relu)

                # pool into batches: q[b, 36] += onehot_t^T @ V, PSUM-accum
                t_oh = wpool.tile([P, gs * P], F32, tag="oh")
                nc.sync.dma_start(t_oh[:], d_oh[:, t0 * P:(t0 + gs) * P])
                for st in range(gs):
                    t = t0 + st
                    nc.tensor.matmul(
                        ps_q[:], lhsT=t_oh[:, st * P:(st + 1) * P],
                        rhs=tV[:, st * HC:(st + 1) * HC],
                        start=(t == 0), stop=(t == n_dst_tiles - 1))

            for idx in range(len(groups)):
                emit_A(idx)
                if idx > 0:
                    emit_B(idx - 1)
            emit_B(len(groups) - 1)

            t_q = cpool.tile([P, HC], F32)
            nc.vector.tensor_copy(t_q[:], ps_q[:])
            nc.sync.dma_start(d_q[:], t_q[:])
    nc.finalize()
    _nc_cache[key] = nc
    return nc


def kernel(**inputs):
    x_s = np.asarray(inputs["x_s"], np.float32)
    x_t = np.asarray(inputs["x_t"], np.float32)
    edge_index = np.asarray(inputs["edge_index"])
    x_s_batch = np.asarray(inputs["x_s_batch"]).astype(np.int64)
    W = np.asarray(inputs["W"], np.float32)
    att_src = np.asarray(inputs["att_src"], np.float32)
    att_dst = np.asarray(inputs["att_dst"], np.float32)
    bias = np.asarray(inputs["bias"], np.float32)
    fc1_w = np.asarray(inputs["fc1_w"], np.float32)
    fc1_b = np.asarray(inputs["fc1_b"], np.float32)
    fc3_w = np.asarray(inputs["fc3_w"], np.float32)
    fc3_b = np.asarray(inputs["fc3_b"], np.float32)

    n_nodes, in_dim = x_s.shape
    src = edge_index[0].astype(np.int64)
    dst = edge_index[1].astype(np.int64)

    # ---- host: edge bucketing by destination (index/layout prep only) ----
    deg = np.bincount(dst, minlength=n_nodes)
    order = np.argsort(-deg, kind="stable")          # nodes by degree desc
    inv_order = np.empty(n_nodes, np.int64)
    inv_order[order] = np.arange(n_nodes)
    nodes_per_core = (n_nodes + N_CORES - 1) // N_CORES
    n_dst_tiles = (nodes_per_core + P - 1) // P
    n_dst_tiles = (n_dst_tiles + 3) // 4 * 4         # whole groups of 4
    L_list = []
    for t in range(n_dst_tiles):
        r0 = t * P * N_CORES
        L = max(4, int(deg[order[min(r0, n_nodes - 1)]]))
        L_list.append((L + 3) // 4 * 4)              # multiple of 4
    k = 0                                            # solo tiles (big L)
    while k < n_dst_tiles and L_list[k] > 44:
        k += 1
    k = min((k + 3) // 4 * 4, n_dst_tiles)
    groups = []
    for t in range(0, k, 2):                         # pair the big-L tiles
        Lg = max(L_list[t:t + 2])
        L_list[t] = L_list[t + 1] = Lg
        groups.append((t, 2))
    for g in range(k, n_dst_tiles, 4):
        Lg = max(L_list[g:g + 4])                    # shared within group
        for t in range(g, g + 4):
            L_list[t] = Lg
        groups.append((g, 4))
    groups = tuple(groups)
    off_arr = np.concatenate([[0], np.cumsum(L_list)]).astype(np.int64)
    slot_tot = int(off_arr[-1])
    half_tot = slot_tot // 2
    n_xt_cols = n_dst_tiles * P

    # edges sorted by dst -> per-node contiguous src runs
    e_order = np.argsort(dst, kind="stable")
    dst_sorted = dst[e_order]
    src_sorted = src[e_order].astype(np.int64)
    starts = np.searchsorted(dst_sorted, np.arange(n_nodes))
    slot_within = np.arange(len(dst_sorted)) - starts[dst_sorted]

    k_global = inv_order[dst_sorted]
    core_of = (k_global % N_CORES).astype(np.int64)
    k_local = k_global // N_CORES
    t_of = k_local // P
    p_of = k_local % P
    col_of = off_arr[t_of] + slot_within

    # fold weights (host weight prep).  W/bias/w2 columns permuted c-major:
    # folded col (c*HEADS + h) <- original col (h*CH + c).
    cm = np.array([h * CH + c for c in range(CH) for h in range(HEADS)])
    W_cm = W[:, cm]
    bias_cm = bias[cm]
    w2_cm = (fc1_w @ fc3_w)[:, 0].astype(np.float32)[cm]

    wa_t = np.einsum("khc,hc->kh", W.reshape(in_dim, HEADS, CH), att_dst)
    wa_s = np.einsum("khc,hc->kh", W.reshape(in_dim, HEADS, CH), att_src)
    wfold = np.zeros((in_dim, ROW), np.float32)
    wfold[:, :HC] = W_cm
    wfold[:, HC:HC + HEADS] = wa_s
    wfbd = np.zeros((2 * in_dim, ROW2), np.float32)
    wfbd[:in_dim, :ROW] = wfold
    wfbd[in_dim:, ROW:] = wfold
    wfbd = wfbd.astype(ml_dtypes.bfloat16)
    wat = np.zeros((in_dim, 4), np.float32)
    wat[:, :HEADS] = wa_t
    biasb = np.tile(bias_cm[None, :], (P, 1)).astype(np.float32)

    xsb_ext = np.zeros((n_nodes + 1, in_dim), ml_dtypes.bfloat16)
    xsb_ext[:n_nodes] = x_s.astype(ml_dtypes.bfloat16)
    SENT = n_nodes

    in_maps = []
    cnts = []
    for c in range(N_CORES):
        node_ids = order[c::N_CORES]                 # this core's dst nodes
        ncnt = len(node_ids)
        m = core_of == c
        SRC = np.full((P, slot_tot), SENT, np.int64)
        SRC[p_of[m], col_of[m]] = src_sorted[m]

        # per-edge lhsT layout: rows 0:64 even slots, 64:128 odd slots
        xe = np.empty((2 * in_dim, half_tot * P), ml_dtypes.bfloat16)
        for par in range(2):
            S = SRC[:, par::2]                       # [P, half_tot]
            blk = xsb_ext[S]                         # [P, half_tot, in_dim]
            xe[par * in_dim:(par + 1) * in_dim] = (
                blk.transpose(2, 1, 0).reshape(in_dim, half_tot * P))

        padc4 = np.zeros((P, n_dst_tiles * 4), np.float32)
        oh = np.zeros((P, n_dst_tiles * P), np.float32)
        xt_t = np.zeros((in_dim, n_xt_cols), np.float32)
        kk = np.arange(n_dst_tiles * P)
        tt, pp = kk // P, kk % P
        present = kk < ncnt
        nid = np.where(present, node_ids[np.minimum(kk, ncnt - 1)], 0)
        Leff = np.asarray(L_list, np.float32)[tt]
        pc = np.where(present, Leff - deg[nid], Leff)
        for j in range(4):
            padc4[pp, 4 * tt + j] = pc
        bid = x_s_batch[nid]
        oh[pp[present], tt[present] * P + bid[present]] = 1.0
        cnts.append(np.bincount(bid[present], minlength=P).astype(np.float64))
        xt_t[:, :ncnt] = x_t[node_ids].T
        in_maps.append({
            "xe": xe, "xt_t": xt_t, "padc4": padc4, "oh": oh, "wfbd": wfbd,
            "wat": wat, "biasb": biasb,
        })

    nc = _build_nc(in_dim, n_dst_tiles, L_list, half_tot, n_xt_cols, groups)
    res = run_bass_kernel_spmd(nc, in_maps, core_ids=list(range(N_CORES)))

    q = np.zeros((P, HC), np.float64)
    cnt = np.zeros(P, np.float64)
    for c in range(N_CORES):
        q += res.results[c]["q_out"]
        cnt += cnts[c]
    num = q @ w2_cm.astype(np.float64)
    out = num / np.maximum(cnt, 1.0)
    const = float(fc1_b @ fc3_w[:, 0] + fc3_b[0])
    return (out + const).astype(np.float32)


# revision 46
# speedup vs baseline: 1.0860x; 1.0860x over previous
"""GAT (bipartite GATConv + mean-pool + 2 FC) on 8 Trainium2 NeuronCores.

Strategy: edges are sharded per destination node; destination nodes are
dealt round-robin (degree-sorted) across the 8 cores so the segment softmax
is fully local to a core.  Per the sharding hint each device holds its edge
shard with the source-node features replicated into matmul-ready per-edge
layout (host does only index manipulation / np.take layout; every model
FLOP runs on device):

  Phase A2: a_t = x_t @ (W att_dst) for this core's dst nodes (PE).
  Phase B: dst nodes are processed in tiles of 128 (one node per partition,
           nodes degree-sorted so tiles have uniform run lengths L).  The
           per-edge source features arrive as bf16 lhsT tiles [128, L/2*128]
           (slot pair 2j/2j+1 stacked as two K=64 halves); one K=128 matmul
           against a block-diagonal [wfold|0 / 0|wfold] rhs computes BOTH
           slots' rows [h_s (36, c-major) | a_s (3) | pad] into PSUM.  ACT
           casts them to bf16 SBUF; E = exp(leaky_relu(a_s+a_t)) =
           max(exp(z), exp(0.2 z)) via two ACT Exp passes over a DVE-added
           z.  The weighted message sum runs in bf16 on DVE's packed 2x
           path: W columns are stored c-major/h-minor so the E broadcast's
           innermost dim is the packed head dim, and the slot reduction is
           two packed tree-add levels plus a short tensor_reduce tail.
           Batch pooling is a PE matmul against host-shipped one-hot
           columns, accumulated over all tiles in PSUM; the final fc1@fc3
           contraction and count division happen on host partials.
           Pad slots carry x=0 => h=0, a_s=0; their exp(leaky_relu(a_t))
           denominator contribution is subtracted exactly via a
           host-precomputed pad-count correction.

Device-side per-edge gathers are avoided entirely: one [P,1]-offset
indirect-DMA gather costs ~1.1us of SWDGE descriptor generation on the Pool
engine (994ns fixed + 0.34ns/desc, 128 descriptors max per instruction) and
the batched-gather ucode (InstDMAGatherAnt etc.) is excluded from bedrock
images, so any gather-based design is floored at ~3.3ms/core.  Sequential
streaming of the pre-laid-out edge shard runs at DMA bandwidth instead.

HW pitfalls (probed): matmuls that switch PE row groups (partition-offset
lhsT/rhs) within one PSUM tile crash the device (the K=128 block-diagonal
formulation sidesteps row groups); Pool-engine TensorTensor is rejected by
this lowering; softmax denominators accumulated from bf16 exps lose ~6x
final accuracy (E stays f32; bf16 is cast only for the message multiply).
Measurement note: identical binaries measured 320.7us and ~386us in
different windows (device clock/p-state or assignment drift, not jitter —
consecutive runs agree within ~1us); single-run deltas below ~15% are not
attributable to code changes.
"""

import numpy as np
import ml_dtypes

import concourse.bacc as bacc
import concourse.tile as tile
from concourse import mybir
from concourse.bass_utils import run_bass_kernel_spmd

F32 = mybir.dt.float32
BF16 = mybir.dt.bfloat16

N_CORES = 8
P = 128
HEADS = 3
CH = 12
HC = HEADS * CH          # 36
ROW = HC + HEADS         # matmul output row: 36 h | 3 a_s = 39
ROW2 = 2 * ROW           # block-diagonal pair output
NEG_SLOPE = 0.2
GP = 6                   # slot pairs per PSUM tile (6*78 = 468 f32 <= 512)

_nc_cache = {}


def _build_nc(in_dim, n_dst_tiles, L_list, half_tot, n_xt_cols, groups):
    key = (in_dim, n_dst_tiles, tuple(L_list), half_tot, n_xt_cols, tuple(groups))
    if key in _nc_cache:
        return _nc_cache[key]

    nc = bacc.Bacc("TRN2", target_bir_lowering=False, debug=False)
    d_xe = nc.dram_tensor("xe", [2 * in_dim, half_tot * P], BF16, kind="ExternalInput")
    d_xt = nc.dram_tensor("xt_t", [in_dim, n_xt_cols], F32, kind="ExternalInput")
    d_pc = nc.dram_tensor("padc4", [P, n_dst_tiles * 4], F32, kind="ExternalInput")
    d_oh = nc.dram_tensor("oh", [P, n_dst_tiles * P], F32, kind="ExternalInput")
    d_wf = nc.dram_tensor("wfbd", [2 * in_dim, ROW2], BF16, kind="ExternalInput")
    d_wt = nc.dram_tensor("wat", [in_dim, 4], F32, kind="ExternalInput")
    d_bb = nc.dram_tensor("biasb", [P, HC], F32, kind="ExternalInput")
    d_q = nc.dram_tensor("q_out", [P, HC], F32, kind="ExternalOutput")

    with tile.TileContext(nc) as tc:
        with tc.tile_pool(name="const", bufs=1) as cpool, \
             tc.tile_pool(name="xload", bufs=2) as xpool, \
             tc.tile_pool(name="gat", bufs=3) as gpool, \
             tc.tile_pool(name="work", bufs=3) as wpool, \
             tc.tile_pool(name="msg", bufs=2) as mpool, \
             tc.tile_pool(name="psA", bufs=4, space="PSUM") as psA, \
             tc.tile_pool(name="psB", bufs=1, space="PSUM") as psB, \
             tc.tile_pool(name="psT", bufs=2, space="PSUM") as psT:

            # ---- constants into SBUF ----
            t_wf = cpool.tile([2 * in_dim, ROW2], BF16)
            nc.sync.dma_start(t_wf[:], d_wf[:])
            t_wt = cpool.tile([in_dim, 4], F32)
            nc.sync.dma_start(t_wt[:], d_wt[:])
            t_bb = cpool.tile([P, HC], F32)
            nc.sync.dma_start(t_bb[:], d_bb[:])
            t_pc = cpool.tile([P, n_dst_tiles * 4], F32)
            nc.sync.dma_start(t_pc[:], d_pc[:])

            # ---- phase B: tiles processed in groups sharing L (the few
            # high-degree tiles run solo; the rest in groups of 4).
            # Software-pipelined: group i's DMA/matmul/copy/z/exp stage (A)
            # is emitted before group i-1's softmax/message stage (B), so
            # the in-order DVE stream has group i-1's heavy message work to
            # run while ACT computes group i's exponentials. ----
            ps_q = psB.tile([P, HC], F32, space="PSUM", tag="q")
            state = {}
            off_h = [0]

            def emit_A(idx):
                t0, gs = groups[idx]
                L = L_list[t0]        # shared within group, multiple of 4
                Lh = L // 2
                GL = gs * L
                xe_sb = xpool.tile([2 * in_dim, gs * Lh * P], BF16, tag="xe")
                nc.sync.dma_start(
                    xe_sb[:], d_xe[:, off_h[0] * P:(off_h[0] + gs * Lh) * P])
                off_h[0] += gs * Lh

                # a_t for this group's tiles (interleaved A2; per-group
                # x_t slice so the first group isn't gated on a monolithic
                # x_t load)
                xt_g = wpool.tile([in_dim, gs * P], F32, tag="xt")
                nc.sync.dma_start(xt_g[:], d_xt[:, t0 * P:(t0 + gs) * P])
                t_atg = wpool.tile([P, 4 * gs], F32, tag="at")
                psa2 = psT.tile([P, 4 * gs], F32, space="PSUM", tag="psat")
                for j in range(gs):
                    nc.tensor.matmul(
                        psa2[:, j * 4:(j + 1) * 4],
                        lhsT=xt_g[:, j * P:(j + 1) * P],
                        rhs=t_wt[:], start=True, stop=True)
                nc.scalar.copy(t_atg[:], psa2[:])

                # per-edge rows via PE: one K=128 matmul per slot PAIR
                g = gpool.tile([P, GL * ROW], BF16, tag="G")
                for st in range(gs):
                    for h0 in range(0, Lh, GP):
                        nh = min(GP, Lh - h0)
                        ps = psA.tile([P, GP * ROW2], F32, space="PSUM", tag="psa")
                        for j in range(nh):
                            nc.tensor.matmul(
                                ps[:, j * ROW2:(j + 1) * ROW2],
                                lhsT=xe_sb[:, (st * Lh + h0 + j) * P:
                                           (st * Lh + h0 + j + 1) * P],
                                rhs=t_wf[:],
                                start=True, stop=True)
                        nc.scalar.copy(
                            g[:, (st * Lh + h0) * ROW2:
                              (st * Lh + h0 + nh) * ROW2],
                            ps[:, :nh * ROW2])

                g4 = g[:].rearrange("p (s l c) -> p s l c", s=gs, c=ROW)

                # z = a_s + a_t  (layout (s, l, h)); exps on ACT
                tZ = wpool.tile([P, GL * HEADS], F32, tag="Z")
                Z4 = tZ[:].rearrange("p (s l h) -> p s l h", s=gs, h=HEADS)
                at_b = (t_atg[:].rearrange("p (s h) -> p s h", h=4)
                        [:, :, 0:HEADS]
                        .unsqueeze(2).to_broadcast((P, gs, L, HEADS)))
                nc.vector.tensor_tensor(
                    out=Z4[:], in0=g4[:, :, :, HC:HC + HEADS], in1=at_b,
                    op=mybir.AluOpType.add)
                tE = wpool.tile([P, GL * HEADS], F32, tag="E")
                tT = wpool.tile([P, GL * HEADS], F32, tag="T")
                nc.scalar.activation(
                    tT[:], tZ[:], mybir.ActivationFunctionType.Exp)
                nc.scalar.activation(
                    tE[:], tZ[:], mybir.ActivationFunctionType.Exp,
                    scale=NEG_SLOPE)
                state[idx] = (t0, gs, L, GL, g, tE, tT, t_atg)

            def emit_B(idx):
                t0, gs, L, GL, g, tE, tT, t_atg = state.pop(idx)
                nc.vector.tensor_tensor(
                    out=tE[:], in0=tE[:], in1=tT[:], op=mybir.AluOpType.max)
                tEb = wpool.tile([P, GL * HEADS], BF16, tag="Eb")
                nc.scalar.copy(tEb[:], tE[:])

                # denominators + pad correction + reciprocal
                t_den = wpool.tile([P, 4 * gs], F32, tag="den")
                t_rec = wpool.tile([P, 4 * gs], F32, tag="rec")
                nc.vector.memset(t_den[:], 1.0)
                nc.vector.tensor_reduce(
                    out=t_den[:].rearrange("p (s h) -> p s h", h=4)
                    [:, :, 0:HEADS],
                    in_=tE[:].rearrange("p (s l h) -> p s l h", s=gs, h=HEADS)
                    .transpose([0, 1, 3, 2]),
                    axis=mybir.AxisListType.X, op=mybir.AluOpType.add)
                # pad correction: cor = padc * max(exp(a_t), exp(0.2 a_t))
                t_c2 = wpool.tile([P, 4 * gs], F32, tag="c2")
                t_c3 = wpool.tile([P, 4 * gs], F32, tag="c3")
                nc.vector.tensor_scalar_mul(t_c2[:], t_atg[:], NEG_SLOPE)
                nc.scalar.activation(
                    t_c3[:], t_c2[:], mybir.ActivationFunctionType.Exp)
                nc.scalar.activation(
                    t_c2[:], t_atg[:], mybir.ActivationFunctionType.Exp)
                nc.vector.tensor_tensor(
                    out=t_c2[:], in0=t_c2[:], in1=t_c3[:],
                    op=mybir.AluOpType.max)
                nc.vector.tensor_tensor(
                    out=t_c2[:], in0=t_c2[:],
                    in1=t_pc[:, t0 * 4:(t0 + gs) * 4],
                    op=mybir.AluOpType.mult)
                dv = (t_den[:].rearrange("p (s h) -> p s h", h=4)
                      [:, :, 0:HEADS])
                cview = (t_c2[:].rearrange("p (s h) -> p s h", h=4)
                         [:, :, 0:HEADS])
                nc.vector.tensor_tensor(
                    out=dv, in0=dv, in1=cview, op=mybir.AluOpType.subtract)
                nc.vector.tensor_scalar_max(t_den[:], t_den[:], 1e-30)
                nc.vector.reciprocal(t_rec[:], t_den[:])

                # weighted messages M = e * h (bf16; c-major h block),
                # one multiply for the whole group ((s,l) merged), then two
                # group-wide pairwise tree-add levels + one reduce tail
                tM = mpool.tile([P, GL * HC], BF16, tag="M")
                tU = wpool.tile([P, gs * HC], F32, tag="U")
                M4v = tM[:].rearrange("p (q c h) -> p q c h", c=CH, h=HEADS)
                e_b = (tEb[:].rearrange("p (q h) -> p q h", h=HEADS)
                       .unsqueeze(2).to_broadcast((P, GL, CH, HEADS)))
                gh = (g[:].rearrange("p (q c) -> p q c", c=ROW)[:, :, 0:HC]
                      .rearrange("p q (c h) -> p q c h", h=HEADS))
                nc.vector.tensor_tensor(
                    out=M4v[:], in0=gh, in1=e_b, op=mybir.AluOpType.mult)
                for n in (L // 2, L // 4):
                    sv = tM[:].rearrange("p (s q) -> p s q", s=gs)
                    src = (sv[:, :, :2 * n * HC]
                           .rearrange("p s (n two c) -> p s n two c",
                                      two=2, c=HC))
                    dstv = (sv[:, :, :n * HC]
                            .rearrange("p s (n c) -> p s n c", c=HC))
                    nc.vector.tensor_tensor(
                        out=dstv, in0=src[:, :, :, 0, :],
                        in1=src[:, :, :, 1, :], op=mybir.AluOpType.add)
                n = L // 4
                while n > 3:
                    k = n // 2
                    rem = n - k
                    sv = tM[:].rearrange("p (s q) -> p s q", s=gs)
                    lo = (sv[:, :, :k * HC]
                          .rearrange("p s (n c) -> p s n c", c=HC))
                    hi = (sv[:, :, rem * HC:n * HC]
                          .rearrange("p s (n c) -> p s n c", c=HC))
                    nc.vector.tensor_tensor(
                        out=lo, in0=lo, in1=hi, op=mybir.AluOpType.add)
                    n = rem
                nc.vector.tensor_reduce(
                    out=tU[:].rearrange("p (s c) -> p s c", s=gs),
                    in_=tM[:].rearrange("p (s q) -> p s q", s=gs)
                    [:, :, :n * HC]
                    .rearrange("p s (n c) -> p s n c", c=HC)
                    .transpose([0, 1, 3, 2]),
                    axis=mybir.AxisListType.X, op=mybir.AluOpType.add)

                # V = relu(U / denom + bias); relu on ACT
                tV = wpool.tile([P, gs * HC], F32, tag="V")
                rec_b = (t_rec[:].rearrange("p (s h) -> p s h", h=4)
                         [:, :, 0:HEADS].unsqueeze(2)
                         .to_broadcast((P, gs, CH, HEADS)))
                nc.vector.tensor_tensor(
                    out=tV[:].rearrange("p (s c h) -> p s c h", c=CH, h=HEADS),
                    in0=tU[:].rearrange("p (s c h) -> p s c h", c=CH, h=HEADS),
                    in1=rec_b, op=mybir.AluOpType.mult)
                bb_b = t_bb[:].unsqueeze(1).to_broadcast((P, gs, HC))
                nc.vector.tensor_tensor(
                    out=tV[:].rearrange("p (s c) -> p s c", s=gs),
                    in0=tV[:].rearrange("p (s c) -> p s c", s=gs),
                    in1=bb_b, op=mybir.AluOpType.add)
                nc.scalar.activation(
                    tV[:], tV[:], mybir.ActivationFunctionType.Relu)

                # pool into batches: q[b, 36] += onehot_t^T @ V, PSUM-accum
                t_oh = wpool.tile([P, gs * P], F32, tag="oh")
                nc.sync.dma_start(t_oh[:], d_oh[:, t0 * P:(t0 + gs) * P])
                for st in range(gs):
                    t = t0 + st
                    nc.tensor.matmul(
                        ps_q[:], lhsT=t_oh[:, st * P:(st + 1) * P],
                        rhs=tV[:, st * HC:(st + 1) * HC],
                        start=(t == 0), stop=(t == n_dst_tiles - 1))

            for idx in range(len(groups)):
                emit_A(idx)
                if idx > 0:
                    emit_B(idx - 1)
            emit_B(len(groups) - 1)

            t_q = cpool.tile([P, HC], F32)
            nc.vector.tensor_copy(t_q[:], ps_q[:])
            nc.sync.dma_start(d_q[:], t_q[:])
    nc.finalize()
    _nc_cache[key] = nc
    return nc


def kernel(**inputs):
    x_s = np.asarray(inputs["x_s"], np.float32)
    x_t = np.asarray(inputs["x_t"], np.float32)
    edge_index = np.asarray(inputs["edge_index"])
    x_s_batch = np.asarray(inputs["x_s_batch"]).astype(np.int64)
    W = np.asarray(inputs["W"], np.float32)
    att_src = np.asarray(inputs["att_src"], np.float32)
    att_dst = np.asarray(inputs["att_dst"], np.float32)
    bias = np.asarray(inputs["bias"], np.float32)
    fc1_w = np.asarray(inputs["fc1_w"], np.float32)
    fc1_b = np.asarray(inputs["fc1_b"], np.float32)
    fc3_w = np.asarray(inputs["fc3_w"], np.float32)
    fc3_b = np.asarray(inputs["fc3_b"], np.float32)

    n_nodes, in_dim = x_s.shape
    src = edge_index[0].astype(np.int64)
    dst = edge_index[1].astype(np.int64)

    # ---- host: edge bucketing by destination (index/layout prep only) ----
    deg = np.bincount(dst, minlength=n_nodes)
    order = np.argsort(-deg, kind="stable")          # nodes by degree desc
    inv_order = np.empty(n_nodes, np.int64)
    inv_order[order] = np.arange(n_nodes)
    nodes_per_core = (n_nodes + N_CORES - 1) // N_CORES
    n_dst_tiles = (nodes_per_core + P - 1) // P
    n_dst_tiles = (n_dst_tiles + 3) // 4 * 4         # whole groups of 4
    L_list = []
    for t in range(n_dst_tiles):
        r0 = t * P * N_CORES
        L = max(4, int(deg[order[min(r0, n_nodes - 1)]]))
        L_list.append((L + 3) // 4 * 4)              # multiple of 4
    k = 0                                            # solo tiles (big L)
    while k < n_dst_tiles and L_list[k] > 44:
        k += 1
    k = min((k + 3) // 4 * 4, n_dst_tiles)
    groups = []
    for t in range(0, k, 2):                         # pair the big-L tiles
        Lg = max(L_list[t:t + 2])
        L_list[t] = L_list[t + 1] = Lg
        groups.append((t, 2))
    for g in range(k, n_dst_tiles, 4):
        Lg = max(L_list[g:g + 4])                    # shared within group
        for t in range(g, g + 4):
            L_list[t] = Lg
        groups.append((g, 4))
    groups = tuple(groups)
    off_arr = np.concatenate([[0], np.cumsum(L_list)]).astype(np.int64)
    slot_tot = int(off_arr[-1])
    half_tot = slot_tot // 2
    n_xt_cols = n_dst_tiles * P

    # edges sorted by dst -> per-node contiguous src runs
    e_order = np.argsort(dst, kind="stable")
    dst_sorted = dst[e_order]
    src_sorted = src[e_order].astype(np.int64)
    starts = np.searchsorted(dst_sorted, np.arange(n_nodes))
    slot_within = np.arange(len(dst_sorted)) - starts[dst_sorted]

    k_global = inv_order[dst_sorted]
    core_of = (k_global % N_CORES).astype(np.int64)
    k_local = k_global // N_CORES
    t_of = k_local // P
    p_of = k_local % P
    col_of = off_arr[t_of] + slot_within

    # fold weights (host weight prep).  W/bias/w2 columns permuted c-major:
    # folded col (c*HEADS + h) <- original col (h*CH + c).
    cm = np.array([h * CH + c for c in range(CH) for h in range(HEADS)])
    W_cm = W[:, cm]
    bias_cm = bias[cm]
    w2_cm = (fc1_w @ fc3_w)[:, 0].astype(np.float32)[cm]

    wa_t = np.einsum("khc,hc->kh", W.reshape(in_dim, HEADS, CH), att_dst)
    wa_s = np.einsum("khc,hc->kh", W.reshape(in_dim, HEADS, CH), att_src)
    wfold = np.zeros((in_dim, ROW), np.float32)
    wfold[:, :HC] = W_cm
    wfold[:, HC:] = wa_s
    wfbd = np.zeros((2 * in_dim, ROW2), np.float32)
    wfbd[:in_dim, :ROW] = wfold
    wfbd[in_dim:, ROW:] = wfold
    wfbd = wfbd.astype(ml_dtypes.bfloat16)
    wat = np.zeros((in_dim, 4), np.float32)
    wat[:, :HEADS] = wa_t
    biasb = np.tile(bias_cm[None, :], (P, 1)).astype(np.float32)

    xsb_ext = np.zeros((n_nodes + 1, in_dim), ml_dtypes.bfloat16)
    xsb_ext[:n_nodes] = x_s.astype(ml_dtypes.bfloat16)
    SENT = n_nodes

    in_maps = []
    cnts = []
    for c in range(N_CORES):
        node_ids = order[c::N_CORES]                 # this core's dst nodes
        ncnt = len(node_ids)
        m = core_of == c
        SRC = np.full((P, slot_tot), SENT, np.int64)
        SRC[p_of[m], col_of[m]] = src_sorted[m]

        # per-edge lhsT layout: rows 0:64 even slots, 64:128 odd slots
        xe = np.empty((2 * in_dim, half_tot * P), ml_dtypes.bfloat16)
        for par in range(2):
            S = SRC[:, par::2]                       # [P, half_tot]
            blk = xsb_ext[S]                         # [P, half_tot, in_dim]
            xe[par * in_dim:(par + 1) * in_dim] = (
                blk.transpose(2, 1, 0).reshape(in_dim, half_tot * P))

        padc4 = np.zeros((P, n_dst_tiles * 4), np.float32)
        oh = np.zeros((P, n_dst_tiles * P), np.float32)
        xt_t = np.zeros((in_dim, n_xt_cols), np.float32)
        kk = np.arange(n_dst_tiles * P)
        tt, pp = kk // P, kk % P
        present = kk < ncnt
        nid = np.where(present, node_ids[np.minimum(kk, ncnt - 1)], 0)
        Leff = np.asarray(L_list, np.float32)[tt]
        pc = np.where(present, Leff - deg[nid], Leff)
        for j in range(4):
            padc4[pp, 4 * tt + j] = pc
        bid = x_s_batch[nid]
        oh[pp[present], tt[present] * P + bid[present]] = 1.0
        cnts.append(np.bincount(bid[present], minlength=P).astype(np.float64))
        xt_t[:, :ncnt] = x_t[node_ids].T
        in_maps.append({
            "xe": xe, "xt_t": xt_t, "padc4": padc4, "oh": oh, "wfbd": wfbd,
            "wat": wat, "biasb": biasb,
        })

    nc = _build_nc(in_dim, n_dst_tiles, L_list, half_tot, n_xt_cols, groups)
    res = run_bass_kernel_spmd(nc, in_maps, core_ids=list(range(N_CORES)))

    q = np.zeros((P, HC), np.float64)
    cnt = np.zeros(P, np.float64)
    for c in range(N_CORES):
        q += res.results[c]["q_out"]
        cnt += cnts[c]
    num = q @ w2_cm.astype(np.float64)
    out = num / np.maximum(cnt, 1.0)
    const = float(fc1_b @ fc3_w[:, 0] + fc3_b[0])
    return (out + const).astype(np.float32)


# revision 47
# speedup vs baseline: 1.2932x; 1.1908x over previous
"""GAT (bipartite GATConv + mean-pool + 2 FC) on 8 Trainium2 NeuronCores.

Strategy: edges are sharded per destination node; destination nodes are
dealt round-robin (degree-sorted) across the 8 cores so the segment softmax
is fully local to a core.  Per the sharding hint each device holds its edge
shard with the source-node features replicated into matmul-ready per-edge
layout (host does only index manipulation / np.take layout; every model
FLOP runs on device):

  Phase A2: a_t = x_t @ (W att_dst) for this core's dst nodes (PE).
  Phase B: dst nodes are processed in tiles of 128 (one node per partition,
           nodes degree-sorted so tiles have uniform run lengths L).  The
           per-edge source features arrive as bf16 lhsT tiles [128, L/2*128]
           (slot pair 2j/2j+1 stacked as two K=64 halves); one K=128 matmul
           against a block-diagonal [wfold|0 / 0|wfold] rhs computes BOTH
           slots' rows [h_s (36, c-major) | a_s (3)] into PSUM.  ACT
           casts them to bf16 SBUF; E = exp(leaky_relu(a_s+a_t)) =
           max(exp(z), exp(0.2 z)) via two ACT Exp passes over a DVE-added
           z.  The weighted message sum runs in bf16 on DVE's packed 2x
           path: W columns are stored c-major/h-minor so the E broadcast's
           innermost dim is the packed head dim, and the slot reduction is
           packed tree-add/fold levels plus a <=3-block tensor_reduce tail.
           Batch pooling is a PE matmul against host-shipped one-hot
           columns, accumulated over all tiles in PSUM; the final fc1@fc3
           contraction and count division happen on host partials.
           Pad slots carry x=0 => h=0, a_s=0; their exp(leaky_relu(a_t))
           denominator contribution is subtracted exactly via a
           host-precomputed pad-count correction.

Device-side per-edge gathers are avoided entirely: one [P,1]-offset
indirect-DMA gather costs ~1.1us of SWDGE descriptor generation on the Pool
engine (994ns fixed + 0.34ns/desc, 128 descriptors max per instruction) and
the batched-gather ucode (InstDMAGatherAnt etc.) is excluded from bedrock
images, so any gather-based design is floored at ~3.3ms/core.  Sequential
streaming of the pre-laid-out edge shard runs at DMA bandwidth instead.

HW pitfalls (probed): matmuls that switch PE row groups (partition-offset
lhsT/rhs) within one PSUM tile crash the device (the K=128 block-diagonal
formulation sidesteps row groups); Pool-engine TensorTensor is rejected by
this lowering; softmax denominators accumulated from bf16 exps lose ~6x
final accuracy (E stays f32; bf16 is cast only for the message multiply).
Measurement note: identical binaries measured 320.7us and ~386us in
different windows (device clock/p-state or assignment drift, not jitter —
consecutive runs agree within ~1us); single-run deltas below ~15% are not
attributable to code changes.
"""

import numpy as np
import ml_dtypes

import concourse.bacc as bacc
import concourse.tile as tile
from concourse import mybir
from concourse.bass_utils import run_bass_kernel_spmd

F32 = mybir.dt.float32
BF16 = mybir.dt.bfloat16

N_CORES = 8
P = 128
HEADS = 3
CH = 12
HC = HEADS * CH          # 36
ROW = HC + HEADS         # matmul output row: 36 h | 3 a_s = 39
ROW2 = 2 * ROW           # block-diagonal pair output
NEG_SLOPE = 0.2
GP = 6                   # slot pairs per PSUM tile (6*78 = 468 f32 <= 512)

_nc_cache = {}


def _build_nc(in_dim, n_dst_tiles, L_list, half_tot, n_xt_cols, groups):
    key = (in_dim, n_dst_tiles, tuple(L_list), half_tot, n_xt_cols, tuple(groups))
    if key in _nc_cache:
        return _nc_cache[key]

    nc = bacc.Bacc("TRN2", target_bir_lowering=False, debug=False)
    d_xe = nc.dram_tensor("xe", [2 * in_dim, half_tot * P], BF16, kind="ExternalInput")
    d_xt = nc.dram_tensor("xt_t", [in_dim, n_xt_cols], F32, kind="ExternalInput")
    d_pc = nc.dram_tensor("padc4", [P, n_dst_tiles * 4], F32, kind="ExternalInput")
    d_oh = nc.dram_tensor("oh", [P, n_dst_tiles * P], F32, kind="ExternalInput")
    d_wf = nc.dram_tensor("wfbd", [2 * in_dim, ROW2], BF16, kind="ExternalInput")
    d_wt = nc.dram_tensor("wat", [in_dim, 4], F32, kind="ExternalInput")
    d_bb = nc.dram_tensor("biasb", [P, HC], F32, kind="ExternalInput")
    d_q = nc.dram_tensor("q_out", [P, HC], F32, kind="ExternalOutput")

    with tile.TileContext(nc) as tc:
        with tc.tile_pool(name="const", bufs=1) as cpool, \
             tc.tile_pool(name="xload", bufs=2) as xpool, \
             tc.tile_pool(name="gat", bufs=3) as gpool, \
             tc.tile_pool(name="work", bufs=3) as wpool, \
             tc.tile_pool(name="msg", bufs=2) as mpool, \
             tc.tile_pool(name="psA", bufs=4, space="PSUM") as psA, \
             tc.tile_pool(name="psB", bufs=1, space="PSUM") as psB, \
             tc.tile_pool(name="psT", bufs=2, space="PSUM") as psT:

            # ---- constants into SBUF ----
            t_wf = cpool.tile([2 * in_dim, ROW2], BF16)
            nc.sync.dma_start(t_wf[:], d_wf[:])
            t_wt = cpool.tile([in_dim, 4], F32)
            nc.sync.dma_start(t_wt[:], d_wt[:])
            t_bb = cpool.tile([P, HC], F32)
            nc.sync.dma_start(t_bb[:], d_bb[:])
            t_pc = cpool.tile([P, n_dst_tiles * 4], F32)
            nc.sync.dma_start(t_pc[:], d_pc[:])

            # ---- phase B: tiles processed in groups sharing L (the few
            # high-degree tiles run solo; the rest in groups of 4).
            # Software-pipelined: group i's DMA/matmul/copy/z/exp stage (A)
            # is emitted before group i-1's softmax/message stage (B), so
            # the in-order DVE stream has group i-1's heavy message work to
            # run while ACT computes group i's exponentials. ----
            ps_q = psB.tile([P, HC], F32, space="PSUM", tag="q")
            state = {}
            off_h = [0]

            def emit_A(idx):
                t0, gs = groups[idx]
                L = L_list[t0]        # shared within group, multiple of 4
                Lh = L // 2
                GL = gs * L
                xe_sb = xpool.tile([2 * in_dim, gs * Lh * P], BF16, tag="xe")
                nc.sync.dma_start(
                    xe_sb[:], d_xe[:, off_h[0] * P:(off_h[0] + gs * Lh) * P])
                off_h[0] += gs * Lh

                # a_t for this group's tiles (interleaved A2; per-group
                # x_t slice so the first group isn't gated on a monolithic
                # x_t load)
                xt_g = wpool.tile([in_dim, gs * P], F32, tag="xt")
                nc.sync.dma_start(xt_g[:], d_xt[:, t0 * P:(t0 + gs) * P])
                t_atg = wpool.tile([P, 4 * gs], F32, tag="at")
                psa2 = psT.tile([P, 4 * gs], F32, space="PSUM", tag="psat")
                for j in range(gs):
                    nc.tensor.matmul(
                        psa2[:, j * 4:(j + 1) * 4],
                        lhsT=xt_g[:, j * P:(j + 1) * P],
                        rhs=t_wt[:], start=True, stop=True)
                nc.scalar.copy(t_atg[:], psa2[:])

                # per-edge rows via PE: one K=128 matmul per slot PAIR
                g = gpool.tile([P, GL * ROW], BF16, tag="G")
                for st in range(gs):
                    for h0 in range(0, Lh, GP):
                        nh = min(GP, Lh - h0)
                        ps = psA.tile([P, GP * ROW2], F32, space="PSUM", tag="psa")
                        for j in range(nh):
                            nc.tensor.matmul(
                                ps[:, j * ROW2:(j + 1) * ROW2],
                                lhsT=xe_sb[:, (st * Lh + h0 + j) * P:
                                           (st * Lh + h0 + j + 1) * P],
                                rhs=t_wf[:],
                                start=True, stop=True)
                        nc.scalar.copy(
                            g[:, (st * Lh + h0) * ROW2:
                              (st * Lh + h0 + nh) * ROW2],
                            ps[:, :nh * ROW2])

                g4 = g[:].rearrange("p (s l c) -> p s l c", s=gs, c=ROW)

                # z = a_s + a_t  (layout (s, l, h)); exps on ACT
                tZ = wpool.tile([P, GL * HEADS], F32, tag="Z")
                Z4 = tZ[:].rearrange("p (s l h) -> p s l h", s=gs, h=HEADS)
                at_b = (t_atg[:].rearrange("p (s h) -> p s h", h=4)
                        [:, :, 0:HEADS]
                        .unsqueeze(2).to_broadcast((P, gs, L, HEADS)))
                nc.vector.tensor_tensor(
                    out=Z4[:], in0=g4[:, :, :, HC:HC + HEADS], in1=at_b,
                    op=mybir.AluOpType.add)
                tE = wpool.tile([P, GL * HEADS], F32, tag="E")
                tT = wpool.tile([P, GL * HEADS], F32, tag="T")
                nc.scalar.activation(
                    tT[:], tZ[:], mybir.ActivationFunctionType.Exp)
                nc.scalar.activation(
                    tE[:], tZ[:], mybir.ActivationFunctionType.Exp,
                    scale=NEG_SLOPE)
                state[idx] = (t0, gs, L, GL, g, tE, tT, t_atg)

            def emit_B(idx):
                t0, gs, L, GL, g, tE, tT, t_atg = state.pop(idx)
                nc.vector.tensor_tensor(
                    out=tE[:], in0=tE[:], in1=tT[:], op=mybir.AluOpType.max)
                tEb = wpool.tile([P, GL * HEADS], BF16, tag="Eb")
                nc.scalar.copy(tEb[:], tE[:])

                # denominators + pad correction + reciprocal
                t_den = wpool.tile([P, 4 * gs], F32, tag="den")
                t_rec = wpool.tile([P, 4 * gs], F32, tag="rec")
                nc.vector.memset(t_den[:], 1.0)
                nc.vector.tensor_reduce(
                    out=t_den[:].rearrange("p (s h) -> p s h", h=4)
                    [:, :, 0:HEADS],
                    in_=tE[:].rearrange("p (s l h) -> p s l h", s=gs, h=HEADS)
                    .transpose([0, 1, 3, 2]),
                    axis=mybir.AxisListType.X, op=mybir.AluOpType.add)
                # pad correction: cor = padc * max(exp(a_t), exp(0.2 a_t))
                t_c2 = wpool.tile([P, 4 * gs], F32, tag="c2")
                t_c3 = wpool.tile([P, 4 * gs], F32, tag="c3")
                nc.vector.tensor_scalar_mul(t_c2[:], t_atg[:], NEG_SLOPE)
                nc.scalar.activation(
                    t_c3[:], t_c2[:], mybir.ActivationFunctionType.Exp)
                nc.scalar.activation(
                    t_c2[:], t_atg[:], mybir.ActivationFunctionType.Exp)
                nc.vector.tensor_tensor(
                    out=t_c2[:], in0=t_c2[:], in1=t_c3[:],
                    op=mybir.AluOpType.max)
                nc.vector.tensor_tensor(
                    out=t_c2[:], in0=t_c2[:],
                    in1=t_pc[:, t0 * 4:(t0 + gs) * 4],
                    op=mybir.AluOpType.mult)
                dv = (t_den[:].rearrange("p (s h) -> p s h", h=4)
                      [:, :, 0:HEADS])
                cview = (t_c2[:].rearrange("p (s h) -> p s h", h=4)
                         [:, :, 0:HEADS])
                nc.vector.tensor_tensor(
                    out=dv, in0=dv, in1=cview, op=mybir.AluOpType.subtract)
                nc.vector.tensor_scalar_max(t_den[:], t_den[:], 1e-30)
                nc.vector.reciprocal(t_rec[:], t_den[:])

                # weighted messages M = e * h (bf16; c-major h block),
                # one multiply for the whole group ((s,l) merged), then two
                # group-wide pairwise tree-add levels + one reduce tail
                tM = mpool.tile([P, GL * HC], BF16, tag="M")
                tU = wpool.tile([P, gs * HC], F32, tag="U")
                M4v = tM[:].rearrange("p (q c h) -> p q c h", c=CH, h=HEADS)
                e_b = (tEb[:].rearrange("p (q h) -> p q h", h=HEADS)
                       .unsqueeze(2).to_broadcast((P, GL, CH, HEADS)))
                gh = (g[:].rearrange("p (q c) -> p q c", c=ROW)[:, :, 0:HC]
                      .rearrange("p q (c h) -> p q c h", h=HEADS))
                nc.vector.tensor_tensor(
                    out=M4v[:], in0=gh, in1=e_b, op=mybir.AluOpType.mult)
                for n in (L // 2, L // 4):
                    sv = tM[:].rearrange("p (s q) -> p s q", s=gs)
                    src = (sv[:, :, :2 * n * HC]
                           .rearrange("p s (n two c) -> p s n two c",
                                      two=2, c=HC))
                    dstv = (sv[:, :, :n * HC]
                            .rearrange("p s (n c) -> p s n c", c=HC))
                    nc.vector.tensor_tensor(
                        out=dstv, in0=src[:, :, :, 0, :],
                        in1=src[:, :, :, 1, :], op=mybir.AluOpType.add)
                n = L // 4
                while n > 3:
                    k = n // 2
                    rem = n - k
                    sv = tM[:].rearrange("p (s q) -> p s q", s=gs)
                    lo = (sv[:, :, :k * HC]
                          .rearrange("p s (n c) -> p s n c", c=HC))
                    hi = (sv[:, :, rem * HC:n * HC]
                          .rearrange("p s (n c) -> p s n c", c=HC))
                    nc.vector.tensor_tensor(
                        out=lo, in0=lo, in1=hi, op=mybir.AluOpType.add)
                    n = rem
                nc.vector.tensor_reduce(
                    out=tU[:].rearrange("p (s c) -> p s c", s=gs),
                    in_=tM[:].rearrange("p (s q) -> p s q", s=gs)
                    [:, :, :n * HC]
                    .rearrange("p s (n c) -> p s n c", c=HC)
                    .transpose([0, 1, 3, 2]),
                    axis=mybir.AxisListType.X, op=mybir.AluOpType.add)

                # V = relu(U / denom + bias); relu on ACT
                tV = wpool.tile([P, gs * HC], F32, tag="V")
                rec_b = (t_rec[:].rearrange("p (s h) -> p s h", h=4)
                         [:, :, 0:HEADS].unsqueeze(2)
                         .to_broadcast((P, gs, CH, HEADS)))
                nc.vector.tensor_tensor(
                    out=tV[:].rearrange("p (s c h) -> p s c h", c=CH, h=HEADS),
                    in0=tU[:].rearrange("p (s c h) -> p s c h", c=CH, h=HEADS),
                    in1=rec_b, op=mybir.AluOpType.mult)
                bb_b = t_bb[:].unsqueeze(1).to_broadcast((P, gs, HC))
                nc.vector.tensor_tensor(
                    out=tV[:].rearrange("p (s c) -> p s c", s=gs),
                    in0=tV[:].rearrange("p (s c) -> p s c", s=gs),
                    in1=bb_b, op=mybir.AluOpType.add)
                nc.scalar.activation(
                    tV[:], tV[:], mybir.ActivationFunctionType.Relu)

                # pool into batches: q[b, 36] += onehot_t^T @ V, PSUM-accum
                t_oh = wpool.tile([P, gs * P], F32, tag="oh")
                nc.sync.dma_start(t_oh[:], d_oh[:, t0 * P:(t0 + gs) * P])
                for st in range(gs):
                    t = t0 + st
                    nc.tensor.matmul(
                        ps_q[:], lhsT=t_oh[:, st * P:(st + 1) * P],
                        rhs=tV[:, st * HC:(st + 1) * HC],
                        start=(t == 0), stop=(t == n_dst_tiles - 1))

            for idx in range(len(groups)):
                emit_A(idx)
                if idx > 0:
                    emit_B(idx - 1)
            emit_B(len(groups) - 1)

            t_q = cpool.tile([P, HC], F32)
            nc.vector.tensor_copy(t_q[:], ps_q[:])
            nc.sync.dma_start(d_q[:], t_q[:])
    nc.finalize()
    _nc_cache[key] = nc
    return nc


def kernel(**inputs):
    x_s = np.asarray(inputs["x_s"], np.float32)
    x_t = np.asarray(inputs["x_t"], np.float32)
    edge_index = np.asarray(inputs["edge_index"])
    x_s_batch = np.asarray(inputs["x_s_batch"]).astype(np.int64)
    W = np.asarray(inputs["W"], np.float32)
    att_src = np.asarray(inputs["att_src"], np.float32)
    att_dst = np.asarray(inputs["att_dst"], np.float32)
    bias = np.asarray(inputs["bias"], np.float32)
    fc1_w = np.asarray(inputs["fc1_w"], np.float32)
    fc1_b = np.asarray(inputs["fc1_b"], np.float32)
    fc3_w = np.asarray(inputs["fc3_w"], np.float32)
    fc3_b = np.asarray(inputs["fc3_b"], np.float32)

    n_nodes, in_dim = x_s.shape
    src = edge_index[0].astype(np.int64)
    dst = edge_index[1].astype(np.int64)

    # ---- host: edge bucketing by destination (index/layout prep only) ----
    deg = np.bincount(dst, minlength=n_nodes)
    order = np.argsort(-deg, kind="stable")          # nodes by degree desc
    inv_order = np.empty(n_nodes, np.int64)
    inv_order[order] = np.arange(n_nodes)
    nodes_per_core = (n_nodes + N_CORES - 1) // N_CORES
    n_dst_tiles = (nodes_per_core + P - 1) // P
    n_dst_tiles = (n_dst_tiles + 3) // 4 * 4         # whole groups of 4
    L_list = []
    for t in range(n_dst_tiles):
        r0 = t * P * N_CORES
        L = max(4, int(deg[order[min(r0, n_nodes - 1)]]))
        L_list.append((L + 3) // 4 * 4)              # multiple of 4
    k = 0                                            # solo tiles (big L)
    while k < n_dst_tiles and L_list[k] > 44:
        k += 1
    k = min((k + 3) // 4 * 4, n_dst_tiles)
    groups = []
    for t in range(0, k, 2):                         # pair the big-L tiles
        Lg = max(L_list[t:t + 2])
        L_list[t] = L_list[t + 1] = Lg
        groups.append((t, 2))
    for g in range(k, n_dst_tiles, 4):
        Lg = max(L_list[g:g + 4])                    # shared within group
        for t in range(g, g + 4):
            L_list[t] = Lg
        groups.append((g, 4))
    groups = tuple(groups)
    off_arr = np.concatenate([[0], np.cumsum(L_list)]).astype(np.int64)
    slot_tot = int(off_arr[-1])
    half_tot = slot_tot // 2
    n_xt_cols = n_dst_tiles * P

    # edges sorted by dst -> per-node contiguous src runs
    e_order = np.argsort(dst, kind="stable")
    dst_sorted = dst[e_order]
    src_sorted = src[e_order].astype(np.int64)
    starts = np.searchsorted(dst_sorted, np.arange(n_nodes))
    slot_within = np.arange(len(dst_sorted)) - starts[dst_sorted]

    k_global = inv_order[dst_sorted]
    core_of = (k_global % N_CORES).astype(np.int64)
    k_local = k_global // N_CORES
    t_of = k_local // P
    p_of = k_local % P
    col_of = off_arr[t_of] + slot_within

    # fold weights (host weight prep).  W/bias/w2 columns permuted c-major:
    # folded col (c*HEADS + h) <- original col (h*CH + c).
    cm = np.array([h * CH + c for c in range(CH) for h in range(HEADS)])
    W_cm = W[:, cm]
    bias_cm = bias[cm]
    w2_cm = (fc1_w @ fc3_w)[:, 0].astype(np.float32)[cm]

    wa_t = np.einsum("khc,hc->kh", W.reshape(in_dim, HEADS, CH), att_dst)
    wa_s = np.einsum("khc,hc->kh", W.reshape(in_dim, HEADS, CH), att_src)
    wfold = np.zeros((in_dim, ROW), np.float32)
    wfold[:, :HC] = W_cm
    wfold[:, HC:] = wa_s
    wfbd = np.zeros((2 * in_dim, ROW2), np.float32)
    wfbd[:in_dim, :ROW] = wfold
    wfbd[in_dim:, ROW:] = wfold
    wfbd = wfbd.astype(ml_dtypes.bfloat16)
    wat = np.zeros((in_dim, 4), np.float32)
    wat[:, :HEADS] = wa_t
    biasb = np.tile(bias_cm[None, :], (P, 1)).astype(np.float32)

    xsb_ext = np.zeros((n_nodes + 1, in_dim), ml_dtypes.bfloat16)
    xsb_ext[:n_nodes] = x_s.astype(ml_dtypes.bfloat16)
    SENT = n_nodes

    in_maps = []
    cnts = []
    for c in range(N_CORES):
        node_ids = order[c::N_CORES]                 # this core's dst nodes
        ncnt = len(node_ids)
        m = core_of == c
        SRC = np.full((P, slot_tot), SENT, np.int64)
        SRC[p_of[m], col_of[m]] = src_sorted[m]

        # per-edge lhsT layout: rows 0:64 even slots, 64:128 odd slots
        xe = np.empty((2 * in_dim, half_tot * P), ml_dtypes.bfloat16)
        for par in range(2):
            S = SRC[:, par::2]                       # [P, half_tot]
            blk = xsb_ext[S]                         # [P, half_tot, in_dim]
            xe[par * in_dim:(par + 1) * in_dim] = (
                blk.transpose(2, 1, 0).reshape(in_dim, half_tot * P))

        padc4 = np.zeros((P, n_dst_tiles * 4), np.float32)
        oh = np.zeros((P, n_dst_tiles * P), np.float32)
        xt_t = np.zeros((in_dim, n_xt_cols), np.float32)
        kk = np.arange(n_dst_tiles * P)
        tt, pp = kk // P, kk % P
        present = kk < ncnt
        nid = np.where(present, node_ids[np.minimum(kk, ncnt - 1)], 0)
        Leff = np.asarray(L_list, np.float32)[tt]
        pc = np.where(present, Leff - deg[nid], Leff)
        for j in range(4):
            padc4[pp, 4 * tt + j] = pc
        bid = x_s_batch[nid]
        oh[pp[present], tt[present] * P + bid[present]] = 1.0
        cnts.append(np.bincount(bid[present], minlength=P).astype(np.float64))
        xt_t[:, :ncnt] = x_t[node_ids].T
        in_maps.append({
            "xe": xe, "xt_t": xt_t, "padc4": padc4, "oh": oh, "wfbd": wfbd,
            "wat": wat, "biasb": biasb,
        })

    nc = _build_nc(in_dim, n_dst_tiles, L_list, half_tot, n_xt_cols, groups)
    res = run_bass_kernel_spmd(nc, in_maps, core_ids=list(range(N_CORES)))

    q = np.zeros((P, HC), np.float64)
    cnt = np.zeros(P, np.float64)
    for c in range(N_CORES):
        q += res.results[c]["q_out"]
        cnt += cnts[c]
    num = q @ w2_cm.astype(np.float64)
    out = num / np.maximum(cnt, 1.0)
    const = float(fc1_b @ fc3_w[:, 0] + fc3_b[0])
    return (out + const).astype(np.float32)
